# revision 1
# baseline (speedup 1.0000x reference)
"""Trainium2 Bass kernel for nn_EnergyFunction (8-core SPMD).

Reference computation (per batch b):
    Q = features @ Wq;  K = features @ Wk                     # [S, 64]
    scores = (Q @ K.T) / 8 * locality_scale / max(|i-j|, 1)   # [S, S]
    charge = sigmoid(features @ w_charge + b_charge)          # [S]
    energy = -scores * charge_i * charge_j

Sharding: core = (b, i-half). Each of the 8 cores handles one batch b
(= core // 2) and one half of the query rows (i0 = (core % 2) * 2048),
producing a [2048, 4096] block of the [4, 4096, 4096] output.

Device-side plan (per core):
  - Inputs in fp16 (features pre-transposed to [512, S] feature-major on
    the host; projection weights [Wk|w_charge] / [Wq*(-loc/8)|w_charge]).
  - Prelim per 512-col seg: 4 accumulating fp16 matmuls -> psum [65,512]
    (rows 0:64 = X^T, row 64 = charge logits); ACT sigmoid -> charge row;
    ACT copy stages X^T to SBUF (frees psum at ACT pace); gpsimd
    partition_broadcast replicates the charge row; DVE multiply folds it:
    K'^T = K^T * c_j, Q'^T = Q^T * c_i (both written as fp32r).
  - Main loop (16 i-tiles x 4 j-blocks, j-outer): 2x PE matmul fp32r
    [64c,128m,512n] into a 2-bank psum [128, 1024] -> one DVE tensor_mul
    with the fp16 Toeplitz mask band slice -> 512 KB DMA out. K-side
    prelim groups k2..k7 are deferred into the main loop just before the
    j-block that reads them, so output DMA starts as early as possible.
    Mask band: vb2d[p, u] = 1/max(|i_base + 1920 + p - u|, 1)
    (host input [128, 6016]; tile (t, j) uses u0 = 1024 j - 128 t + 1920).
"""

import numpy as np

import concourse.bacc as bacc
import concourse.mybir as mybir
from concourse import tile
from concourse import bass_utils

# Problem shape (hardcoded per harness contract)
B = 4
S = 4096
F = 512
D = 64

P = 128            # partition tile (i)
SEG = 512          # j segment width (one PSUM bank of fp32)
WOUT = 1024        # epilogue / output tile width (2 PSUM banks)
IHALF = S // 2     # 2048 query rows per core
NIT = IHALF // P   # 16 i-tiles
NSEG = S // SEG    # 8 j segments
NJP = S // WOUT    # 4 j output tiles per i-tile
NQSEG = IHALF // SEG  # 4 q segments
NCH = F // P       # 4 feature chunks
C0 = IHALF - P     # 1920 mask-band column offset
MBW = (S - SEG) + C0 + SEG  # 6016 mask band width

F32 = mybir.dt.float32
F32R = mybir.dt.float32r
F16 = mybir.dt.float16
SIG = mybir.ActivationFunctionType.Sigmoid
COPY = mybir.ActivationFunctionType.Copy

_PROGRAM = None


def _build_program():
    nc = bacc.Bacc("TRN2", target_bir_lowering=False, debug=False, num_devices=8)

    fK = nc.dram_tensor("fK", [F, S], F16, kind="ExternalInput").ap()
    fQ = nc.dram_tensor("fQ", [F, IHALF], F16, kind="ExternalInput").ap()
    # [Wk | w_charge] and [Wq * (-loc/8) | w_charge], both [F, 65]
    wk65 = nc.dram_tensor("wk65", [F, D + 1], F16, kind="ExternalInput").ap()
    wq65 = nc.dram_tensor("wq65", [F, D + 1], F16, kind="ExternalInput").ap()
    bvec = nc.dram_tensor("bvec", [P, 1], F32, kind="ExternalInput").ap()
    vb2d = nc.dram_tensor("vb2d", [P, MBW], F16, kind="ExternalInput").ap()
    energy = nc.dram_tensor("energy", [IHALF, S], F32, kind="ExternalOutput").ap()

    W65 = D + 1
    NSH = WOUT // SEG      # matmul halves per output tile
    VBC = 4                # mask band load chunks
    VBW = MBW // VBC       # 1504

    with tile.TileContext(nc) as tc:
        with (
            tc.tile_pool(name="const", bufs=1) as const,
            tc.tile_pool(name="stage", bufs=1) as stage,
        ):
            bvec_sb = const.tile([P, 1], F32, tag="bvec")
            nc.sync.dma_start(out=bvec_sb[:], in_=bvec)
            wk_sb = const.tile([P, NCH * W65], F16, tag="wk")
            wq_sb = const.tile([P, NCH * W65], F16, tag="wq")
            for c in range(NCH):
                nc.sync.dma_start(
                    out=wk_sb[:, c * W65:(c + 1) * W65],
                    in_=wk65[c * P:(c + 1) * P, :],
                )
                nc.sync.dma_start(
                    out=wq_sb[:, c * W65:(c + 1) * W65],
                    in_=wq65[c * P:(c + 1) * P, :],
                )

            # Persistent prelim outputs
            QT = stage.tile([D, IHALF], F32R, tag="qt")    # Q^T * c_i
            KpT = stage.tile([D, S], F32R, tag="kpt")      # K^T * c_j
            crow = stage.tile([1, S], F32, tag="crow")     # K-side charge row
            qrow = stage.tile([1, IHALF], F32, tag="qrow")  # Q-side charge row
            vb_sb = stage.tile([P, MBW], F16, tag="vb")

            with (
                tc.tile_pool(name="feat", bufs=1) as fpool,
                tc.tile_pool(name="pp", space="PSUM", bufs=2) as ps_p,
            ):
                # Loads in consumption order: fQ first half (q0/q1 segs),
                # the fK column block for k0/k1, fQ second half, then the
                # remaining fK blocks and mask chunks interleaved so the
                # DMA stream delivers each prelim group's data just ahead
                # of its consumers.
                fk = [fpool.tile([P, S], F16, tag=f"fk{c}", name=f"fkt{c}")
                      for c in range(NCH)]
                fq = [fpool.tile([P, IHALF], F16, tag=f"fq{c}", name=f"fqt{c}")
                      for c in range(NCH)]

                def _load_fq_half(half):
                    lo, hi = half * (IHALF // 2), (half + 1) * (IHALF // 2)
                    for c in range(NCH):
                        nc.sync.dma_start(
                            out=fq[c][:, lo:hi], in_=fQ[c * P:(c + 1) * P, lo:hi]
                        )

                def _load_fk_block(b):
                    lo, hi = b * 1024, (b + 1) * 1024
                    for c in range(NCH):
                        nc.sync.dma_start(
                            out=fk[c][:, lo:hi], in_=fK[c * P:(c + 1) * P, lo:hi]
                        )

                def _load_vb(v):
                    nc.sync.dma_start(
                        out=vb_sb[:, v * VBW:(v + 1) * VBW],
                        in_=vb2d[:, v * VBW:(v + 1) * VBW],
                    )

                _load_fk_block(0)
                _load_fq_half(0)
                _load_fq_half(1)
                for b in range(1, S // 1024):
                    _load_fk_block(b)
                for v in (1, 0, 2, 3):
                    _load_vb(v)

                # Per-seg projection chain: 4 accumulating matmuls ->
                # ACT sigmoid (charge row) + ACT copy (frees the psum slot
                # at ACT pace) -> gpsimd broadcast -> DVE fold multiply
                # (writes fp32r Q'/K'). No PE work after the matmuls, so
                # the chain is emitted inline.
                def _emit_bcast_fold(xs, side, s):
                    row = crow if side == "k" else qrow
                    dst = KpT if side == "k" else QT
                    Cb = stage.tile([D, SEG], F32, tag="cb", bufs=2)
                    nc.gpsimd.partition_broadcast(
                        Cb[:], row[0:1, s * SEG:(s + 1) * SEG]
                    )
                    nc.vector.tensor_mul(
                        out=dst[:, s * SEG:(s + 1) * SEG],
                        in0=xs[:],
                        in1=Cb[:],
                    )

                def _emit_group(side, s):
                    w_sb = wk_sb if side == "k" else wq_sb
                    f_t = fk if side == "k" else fq
                    row = crow if side == "k" else qrow
                    pX = ps_p.tile([W65, SEG], F32, tag="pp")
                    for c in range(NCH):
                        nc.tensor.matmul(
                            pX[:],
                            w_sb[:, c * W65:(c + 1) * W65],
                            f_t[c][:, s * SEG:(s + 1) * SEG],
                            start=(c == 0),
                            stop=(c == NCH - 1),
                        )
                    nc.scalar.activation(
                        row[0:1, s * SEG:(s + 1) * SEG], pX[D:D + 1, :],
                        SIG, bias=bvec_sb[0:1, :], scale=1.0,
                    )
                    # stage the projection rows out of PSUM on the (idle)
                    # scalar engine so the psum slot frees at ACT pace and
                    # the PE never throttles on the fold chain
                    xs = stage.tile([D, SEG], F32, tag="xs", bufs=3)
                    nc.scalar.activation(xs[:], pX[0:D, :], COPY)
                    _emit_bcast_fold(xs, side, s)

                # Only the prelim groups the first output block needs run
                # up front (k0/k1 for j=0 plus the whole Q side); the
                # remaining K groups are deferred into the main loop just
                # before the j-block that reads them, so the main loop
                # starts as soon as the fQ-side input lands.
                _emit_group("k", 0)
                _emit_group("k", 1)
                for s in range(NQSEG):
                    _emit_group("q", s)

                with (
                    tc.tile_pool(name="pse", space="PSUM", bufs=3) as ps_e,
                    tc.tile_pool(name="osb", bufs=4) as opool,
                ):
                    for j in range(NJP):
                        if j > 0:
                            _emit_group("k", 2 * j)
                            _emit_group("k", 2 * j + 1)
                        for t in range(NIT):
                            pe_ = ps_e.tile([P, WOUT], F32)
                            for h in range(NSH):
                                nc.tensor.matmul(
                                    pe_[:, h * SEG:(h + 1) * SEG],
                                    QT[:, t * P:(t + 1) * P],
                                    KpT[:, (NSH * j + h) * SEG:
                                        (NSH * j + h + 1) * SEG],
                                    start=True,
                                    stop=True,
                                )
                            osb = opool.tile([P, WOUT], F32)
                            u0 = j * WOUT - t * P + C0
                            nc.vector.tensor_mul(
                                out=osb[:], in0=pe_[:],
                                in1=vb_sb[:, u0:u0 + WOUT],
                            )
                            nc.sync.dma_start(
                                out=energy[t * P:(t + 1) * P,
                                           j * WOUT:(j + 1) * WOUT],
                                in_=osb[:],
                            )

    nc.compile()
    return nc


def _get_program():
    global _PROGRAM
    if _PROGRAM is None:
        _PROGRAM = _build_program()
    return _PROGRAM


def _make_in_maps(features, Wq, Wk, w_charge, b_charge, loc):
    wq_s = Wq * np.float32(-loc / 8.0)
    wq65 = np.ascontiguousarray(
        np.concatenate([wq_s, w_charge[:, None]], axis=1).astype(np.float16)
    )
    wk65 = np.ascontiguousarray(
        np.concatenate([Wk, w_charge[:, None]], axis=1).astype(np.float16)
    )
    bvec = np.full((P, 1), b_charge, dtype=np.float32)

    u = np.arange(MBW, dtype=np.float32)[None, :]
    vb_half = []
    for h in range(2):
        ib = (h * IHALF + C0 + np.arange(P, dtype=np.float32))[:, None]
        vb_half.append(np.ascontiguousarray(
            (1.0 / np.maximum(np.abs(ib - u), 1.0)).astype(np.float16)
        ))

    fT = [np.ascontiguousarray(features[b].T.astype(np.float16)) for b in range(B)]

    in_maps = []
    for core in range(2 * B):
        b, h = divmod(core, 2)
        i0 = h * IHALF
        in_maps.append({
            "fK": fT[b],
            "fQ": np.ascontiguousarray(fT[b][:, i0:i0 + IHALF]),
            "wk65": wk65,
            "wq65": wq65,
            "bvec": bvec,
            "vb2d": vb_half[h],
        })
    return in_maps


def kernel(features, Wq, Wk, w_charge, b_charge, locality_scale):
    features = np.asarray(features, dtype=np.float32)
    Wq = np.asarray(Wq, dtype=np.float32)
    Wk = np.asarray(Wk, dtype=np.float32)
    w_charge = np.asarray(w_charge, dtype=np.float32)
    b_charge = float(np.asarray(b_charge))
    loc = float(np.asarray(locality_scale))

    nc = _get_program()
    in_maps = _make_in_maps(features, Wq, Wk, w_charge, b_charge, loc)
    res = bass_utils.run_bass_kernel_spmd(nc, in_maps, core_ids=list(range(2 * B)))

    out = np.empty((B, S, S), dtype=np.float32)
    for core in range(2 * B):
        b, h = divmod(core, 2)
        out[b, h * IHALF:(h + 1) * IHALF, :] = res.results[core]["energy"]
    return out



# revision 2
# speedup vs baseline: 1.3902x; 1.3902x over previous
"""Trainium2 Bass kernel for nn_EnergyFunction (8-core SPMD), v2.

Reference computation (per batch b):
    Q = features @ Wq;  K = features @ Wk                     # [S, 64]
    scores = (Q @ K.T) / 8 * locality_scale / max(|i-j|, 1)   # [S, S]
    charge = sigmoid(features @ w_charge + b_charge)          # [S]
    energy = -scores * charge_i * charge_j

v2 design: the device computes G[i, j] = -(loc/8) * (Q_i . K_j) * c_i * c_j
in fp16 (WITHOUT the 1/dist locality mask); the host applies the exact
Toeplitz mask in fp32 while unsharding. This halves output HBM traffic
(fp16 vs fp32) and removes the mask table input + the DVE mask multiply.

Sharding: core = (b, i-half): b = core // 2, i0 = (core % 2) * 2048.
Column-permuted frame per core: G cols [0, 2048) are the "own" j-half
[i0, i0+2048) and cols [2048, 4096) the other half, so one shared SPMD
program can read Q-features from the same SBUF tiles as the own-half
K-features (features are supplied as two [512, 2048] halves fA/fB and
not duplicated). The host un-permutes columns during assembly.

Device plan (per core):
  - Prelim per 512-col seg: 4 accumulating fp16 matmuls -> psum [65, 512]
    fp32 (rows 0:64 = X^T, row 64 = charge logits for K-side); ACT
    sigmoid -> charge row; gpsimd partition_broadcast replicates it; DVE
    multiply folds it producing fp16 K'^T = K^T * c_j and Q'^T = Q^T *
    c_i * (-loc/8). Own-half charge (cols 0..2047) serves both c_j and
    c_i, so Q-side groups skip the charge row and reuse the broadcast.
  - Main loop, j-outer (4 j-blocks of 1024 x 16 i-tiles): 2 fp16 matmuls
    [64c, 128m, 512n] -> psum [128, 1024] fp32; drained to fp16 SBUF
    alternating ScalarE ACT-copy / VectorE tensor_copy by i-tile parity
    (either engine alone is slower than the PE); 256 KB DMA out.
  - K-side prelim groups 4..7 (other half, fB) are deferred until just
    before the j-block that reads them so the main loop starts as soon
    as the own-half features land.
"""

import numpy as np

import concourse.bacc as bacc
import concourse.mybir as mybir
from concourse import tile
from concourse import bass_utils

# Problem shape (hardcoded per harness contract)
B = 4
S = 4096
F = 512
D = 64

P = 128            # partition tile (i)
SEG = 512          # j segment width (one PSUM bank of fp32)
WOUT = 1024        # drain / output tile width (2 PSUM banks)
IHALF = S // 2     # 2048 query rows per core
NIT = IHALF // P   # 16 i-tiles
NSEG = S // SEG    # 8 j segments
NJP = S // WOUT    # 4 j output blocks
NCH = F // P       # 4 feature chunks
HCOL = IHALF // 2  # 1024: feature-load column half

F32 = mybir.dt.float32
F16 = mybir.dt.float16
SIG = mybir.ActivationFunctionType.Sigmoid
COPY = mybir.ActivationFunctionType.Copy

_PROGRAM = None


def _build_program():
    nc = bacc.Bacc("TRN2", target_bir_lowering=False, debug=False, num_devices=8)

    fA = nc.dram_tensor("fA", [F, IHALF], F16, kind="ExternalInput").ap()
    fB = nc.dram_tensor("fB", [F, IHALF], F16, kind="ExternalInput").ap()
    # [Wk | w_charge] ([F, 65]) and Wq * (-loc/8) ([F, 64])
    wk65 = nc.dram_tensor("wk65", [F, D + 1], F16, kind="ExternalInput").ap()
    wq64 = nc.dram_tensor("wq64", [F, D], F16, kind="ExternalInput").ap()
    bvec = nc.dram_tensor("bvec", [P, 1], F32, kind="ExternalInput").ap()
    G = nc.dram_tensor("G", [IHALF, S], F16, kind="ExternalOutput").ap()

    W65 = D + 1

    with tile.TileContext(nc) as tc:
        with (
            tc.tile_pool(name="const", bufs=1) as const,
            tc.tile_pool(name="stage", bufs=1) as stage,
        ):
            bvec_sb = const.tile([P, 1], F32, tag="bvec")
            nc.sync.dma_start(out=bvec_sb[:], in_=bvec)
            wk_sb = const.tile([P, NCH * W65], F16, tag="wk")
            wq_sb = const.tile([P, NCH * D], F16, tag="wq")
            for c in range(NCH):
                nc.sync.dma_start(
                    out=wk_sb[:, c * W65:(c + 1) * W65],
                    in_=wk65[c * P:(c + 1) * P, :],
                )
                nc.sync.dma_start(
                    out=wq_sb[:, c * D:(c + 1) * D],
                    in_=wq64[c * P:(c + 1) * P, :],
                )

            # Persistent prelim outputs
            QT = stage.tile([D, IHALF], F16, tag="qt")     # Q^T * c_i * (-loc/8)
            KpT = stage.tile([D, S], F16, tag="kpt")       # K^T * c_j
            crow = stage.tile([1, S], F32, tag="crow")     # charge row (permuted cols)

            with (
                tc.tile_pool(name="feat", bufs=1) as fpool,
                tc.tile_pool(name="pp", space="PSUM", bufs=2) as ps_p,
            ):
                fa = [fpool.tile([P, IHALF], F16, tag=f"fa{c}", name=f"fat{c}")
                      for c in range(NCH)]
                fb = [fpool.tile([P, IHALF], F16, tag=f"fb{c}", name=f"fbt{c}")
                      for c in range(NCH)]

                # Loads in consumption order: own half (fA) by column half
                # so the first prelim groups start early, then fB.
                for half in range(2):
                    lo, hi = half * HCOL, (half + 1) * HCOL
                    for c in range(NCH):
                        nc.sync.dma_start(
                            out=fa[c][:, lo:hi], in_=fA[c * P:(c + 1) * P, lo:hi]
                        )
                for half in range(2):
                    lo, hi = half * HCOL, (half + 1) * HCOL
                    for c in range(NCH):
                        nc.sync.dma_start(
                            out=fb[c][:, lo:hi], in_=fB[c * P:(c + 1) * P, lo:hi]
                        )

                def _k_group(s):
                    """K-side prelim for permuted seg s: K'^T seg + charge."""
                    f_t = fa if s < NSEG // 2 else fb
                    ls = s % (NSEG // 2)
                    pX = ps_p.tile([W65, SEG], F32, tag="pp")
                    for c in range(NCH):
                        nc.tensor.matmul(
                            pX[:],
                            wk_sb[:, c * W65:(c + 1) * W65],
                            f_t[c][:, ls * SEG:(ls + 1) * SEG],
                            start=(c == 0),
                            stop=(c == NCH - 1),
                        )
                    nc.scalar.activation(
                        crow[0:1, s * SEG:(s + 1) * SEG], pX[D:D + 1, :],
                        SIG, bias=bvec_sb[0:1, :], scale=1.0,
                    )
                    Cb = stage.tile([D, SEG], F32, tag="cb", bufs=2)
                    nc.gpsimd.partition_broadcast(
                        Cb[:], crow[0:1, s * SEG:(s + 1) * SEG]
                    )
                    nc.vector.tensor_mul(
                        out=KpT[:, s * SEG:(s + 1) * SEG],
                        in0=pX[0:D, :],
                        in1=Cb[:],
                    )
                    return Cb

                def _q_group(s, Cb):
                    """Q-side prelim for own seg s (charge bcast reused)."""
                    pX = ps_p.tile([W65, SEG], F32, tag="pp")
                    for c in range(NCH):
                        nc.tensor.matmul(
                            pX[0:D, :],
                            wq_sb[:, c * D:(c + 1) * D],
                            fa[c][:, s * SEG:(s + 1) * SEG],
                            start=(c == 0),
                            stop=(c == NCH - 1),
                        )
                    nc.vector.tensor_mul(
                        out=QT[:, s * SEG:(s + 1) * SEG],
                        in0=pX[0:D, :],
                        in1=Cb[:],
                    )

                # Own-half prelims up front (all read fA); each Q group
                # reuses the charge broadcast of its matching K group.
                for s in range(NSEG // 2):
                    Cb = _k_group(s)
                    _q_group(s, Cb)

                with (
                    tc.tile_pool(name="pse", space="PSUM", bufs=3) as ps_e,
                    tc.tile_pool(name="osb", bufs=6) as opool,
                ):
                    for jb in range(NJP):
                        if jb >= 2:
                            _k_group(2 * jb)
                            _k_group(2 * jb + 1)
                        for t in range(NIT):
                            pe_ = ps_e.tile([P, WOUT], F32)
                            for hh in range(2):
                                nc.tensor.matmul(
                                    pe_[:, hh * SEG:(hh + 1) * SEG],
                                    QT[:, t * P:(t + 1) * P],
                                    KpT[:, jb * WOUT + hh * SEG:
                                        jb * WOUT + (hh + 1) * SEG],
                                    start=True,
                                    stop=True,
                                )
                            osb = opool.tile([P, WOUT], F16)
                            if t % 2 == 0:
                                nc.scalar.activation(osb[:], pe_[:], COPY)
                            else:
                                nc.vector.tensor_copy(out=osb[:], in_=pe_[:])
                            nc.sync.dma_start(
                                out=G[t * P:(t + 1) * P,
                                      jb * WOUT:(jb + 1) * WOUT],
                                in_=osb[:],
                            )

    nc.compile()
    return nc


def _get_program():
    global _PROGRAM
    if _PROGRAM is None:
        _PROGRAM = _build_program()
    return _PROGRAM


def _make_in_maps(features, Wq, Wk, w_charge, b_charge, loc):
    wq64 = np.ascontiguousarray(
        (Wq * np.float32(-loc / 8.0)).astype(np.float16)
    )
    wk65 = np.ascontiguousarray(
        np.concatenate([Wk, w_charge[:, None]], axis=1).astype(np.float16)
    )
    bvec = np.full((P, 1), b_charge, dtype=np.float32)

    halves = []
    for b in range(B):
        fT = features[b].T.astype(np.float16)
        halves.append([np.ascontiguousarray(fT[:, h * IHALF:(h + 1) * IHALF])
                       for h in range(2)])

    in_maps = []
    for core in range(2 * B):
        b, h = divmod(core, 2)
        in_maps.append({
            "fA": halves[b][h],
            "fB": halves[b][1 - h],
            "wk65": wk65,
            "wq64": wq64,
            "bvec": bvec,
        })
    return in_maps


def _host_masks():
    """Toeplitz 1/dist blocks: diagonal [2048,2048] and off-diagonal."""
    idx = np.arange(IHALF, dtype=np.float32)
    md = 1.0 / np.maximum(np.abs(idx[:, None] - idx[None, :]), 1.0)
    mo = 1.0 / (np.float32(IHALF) + idx[None, :] - idx[:, None])
    return md.astype(np.float32), mo.astype(np.float32)


def kernel(features, Wq, Wk, w_charge, b_charge, locality_scale):
    features = np.asarray(features, dtype=np.float32)
    Wq = np.asarray(Wq, dtype=np.float32)
    Wk = np.asarray(Wk, dtype=np.float32)
    w_charge = np.asarray(w_charge, dtype=np.float32)
    b_charge = float(np.asarray(b_charge))
    loc = float(np.asarray(locality_scale))

    nc = _get_program()
    in_maps = _make_in_maps(features, Wq, Wk, w_charge, b_charge, loc)
    res = bass_utils.run_bass_kernel_spmd(nc, in_maps, core_ids=list(range(2 * B)))

    md, mo = _host_masks()
    mot = np.ascontiguousarray(mo.T)
    out = np.empty((B, S, S), dtype=np.float32)
    for core in range(2 * B):
        b, h = divmod(core, 2)
        i0 = h * IHALF
        o0 = (1 - h) * IHALF
        Gc = res.results[core]["G"]
        np.multiply(Gc[:, :IHALF], md, out=out[b, i0:i0 + IHALF, i0:i0 + IHALF])
        np.multiply(Gc[:, IHALF:], mo if h == 0 else mot,
                    out=out[b, i0:i0 + IHALF, o0:o0 + IHALF])
    return out


# revision 3
# speedup vs baseline: 1.4219x; 1.0228x over previous
"""Trainium2 Bass kernel for nn_EnergyFunction (8-core SPMD), v3.

Reference computation (per batch b):
    Q = features @ Wq;  K = features @ Wk                     # [S, 64]
    scores = (Q @ K.T) / 8 * locality_scale / max(|i-j|, 1)   # [S, S]
    charge = sigmoid(features @ w_charge + b_charge)          # [S]
    energy = -scores * charge_i * charge_j

The device computes G[i, j] = -(loc/8) * (Q_i . K_j) * c_i * c_j in fp16
(WITHOUT the 1/dist locality mask); the host applies the exact Toeplitz
mask in fp32 while unsharding. This halves output HBM traffic and
removes the mask-table input + the mask multiply from the device.

Sharding: core = (b, i-half): b = core // 2, i0 = (core % 2) * 2048.
Column-permuted frame per core: G cols [0, 2048) are the "own" j-half
[i0, i0+2048) and cols [2048, 4096) the other half, so one shared SPMD
program reads Q-features from the same SBUF tiles as the own-half
K-features (features supplied as two packed [128, 8192] halves, not
duplicated). The host un-permutes columns during assembly.

Device plan (per core):
  - Inputs arrive on the scalar-engine HWDGE ring (separate from the
    sync-engine ring used for output) so feature loads are never queued
    behind output stores: 1 packed weight DMA + 4+4 quarter-column
    feature DMAs of 512 KB.
  - Prelim per 512-col seg: 4 accumulating fp16 matmuls -> psum [65,512]
    fp32 (rows 0:64 = X^T, row 64 = K-side charge logits); ACT sigmoid
    -> charge row; gpsimd partition_broadcast; DVE multiply folds it:
    K'^T = K^T * c_j and Q'^T = Q^T * c_i * (-loc/8), both fp16. Q-side
    groups reuse the own-half charge broadcast (same columns).
  - Main loop, j-outer (4 j-blocks of 1024 x 16 i-tiles): 2 fp16 matmuls
    [64c, 128m, 512n] -> psum [128, 1024] fp32; drained to fp16 SBUF
    alternating ScalarE ACT-copy / VectorE tensor_copy by i-tile parity;
    256 KB DMA out on the sync ring. Prelim groups are interleaved one
    j-phase ahead of use (k4..k7 prefetched during jb=1/2) so the PE
    never idles long enough for the HAM clock gate to re-throttle.
"""

import numpy as np

import concourse.bacc as bacc
import concourse.mybir as mybir
from concourse import tile
from concourse import bass_utils

# Problem shape (hardcoded per harness contract)
B = 4
S = 4096
F = 512
D = 64

P = 128            # partition tile (i)
SEG = 512          # j segment width (one PSUM bank of fp32)
WOUT = 1024        # drain / output tile width (2 PSUM banks)
IHALF = S // 2     # 2048 query rows per core
NIT = IHALF // P   # 16 i-tiles
NSEG = S // SEG    # 8 j segments
NJP = S // WOUT    # 4 j output blocks
NCH = F // P       # 4 feature chunks
FPK = NCH * IHALF  # 8192 packed feature columns per half
WKW = NCH * (D + 1)          # 260
WQW = NCH * D                # 256
WPW = WKW + WQW + 1          # 517 packed weight columns

F32 = mybir.dt.float32
F16 = mybir.dt.float16
SIG = mybir.ActivationFunctionType.Sigmoid
COPY = mybir.ActivationFunctionType.Copy

_PROGRAM = None


def _build_program():
    nc = bacc.Bacc("TRN2", target_bir_lowering=False, debug=False, num_devices=8)

    # packed features: f[p, q*2048 + c*512 + j] = feat[q*512 + j, c*128 + p]
    fA = nc.dram_tensor("fA", [P, FPK], F16, kind="ExternalInput").ap()
    fB = nc.dram_tensor("fB", [P, FPK], F16, kind="ExternalInput").ap()
    # packed weights: [wk(4x65) | wq(4x64) | b_charge]
    wpack = nc.dram_tensor("wpack", [P, WPW], F16, kind="ExternalInput").ap()
    G = nc.dram_tensor("G", [IHALF, S], F16, kind="ExternalOutput").ap()

    W65 = D + 1

    with tile.TileContext(nc) as tc:
        with (
            tc.tile_pool(name="const", bufs=1) as const,
            tc.tile_pool(name="stage", bufs=1) as stage,
        ):
            wp_sb = const.tile([P, WPW], F16, tag="wp")
            nc.scalar.dma_start(out=wp_sb[:], in_=wpack)

            # Persistent prelim outputs
            QT = stage.tile([D, IHALF], F16, tag="qt")     # Q^T * c_i * (-loc/8)
            KpT = stage.tile([D, S], F16, tag="kpt")       # K^T * c_j
            crow = stage.tile([1, S], F32, tag="crow")     # charge row

            with (
                tc.tile_pool(name="feat", bufs=1) as fpool,
                tc.tile_pool(name="pp", space="PSUM", bufs=2) as ps_p,
            ):
                fa_all = fpool.tile([P, FPK], F16, tag="fa")
                fb_all = fpool.tile([P, FPK], F16, tag="fb")
                QW = NCH * SEG  # 2048 packed cols per quarter
                for q in range(4):
                    nc.scalar.dma_start(
                        out=fa_all[:, q * QW:(q + 1) * QW],
                        in_=fA[:, q * QW:(q + 1) * QW],
                    )
                for q in range(4):
                    nc.scalar.dma_start(
                        out=fb_all[:, q * QW:(q + 1) * QW],
                        in_=fB[:, q * QW:(q + 1) * QW],
                    )

                def _k_group(s):
                    """K-side prelim for permuted seg s: K'^T seg + charge."""
                    f_all = fa_all if s < NSEG // 2 else fb_all
                    ls = s % (NSEG // 2)
                    pX = ps_p.tile([W65, SEG], F32, tag="pp")
                    for c in range(NCH):
                        nc.tensor.matmul(
                            pX[:],
                            wp_sb[:, c * W65:(c + 1) * W65],
                            f_all[:, ls * QW + c * SEG:ls * QW + (c + 1) * SEG],
                            start=(c == 0),
                            stop=(c == NCH - 1),
                        )
                    nc.scalar.activation(
                        crow[0:1, s * SEG:(s + 1) * SEG], pX[D:D + 1, :],
                        SIG, bias=wp_sb[0:1, WPW - 1:WPW], scale=1.0,
                    )
                    Cb = stage.tile([D, SEG], F32, tag="cb", bufs=2)
                    nc.gpsimd.partition_broadcast(
                        Cb[:], crow[0:1, s * SEG:(s + 1) * SEG]
                    )
                    nc.vector.tensor_mul(
                        out=KpT[:, s * SEG:(s + 1) * SEG],
                        in0=pX[0:D, :],
                        in1=Cb[:],
                    )
                    return Cb

                def _q_group(s, Cb):
                    """Q-side prelim for own seg s (charge bcast reused)."""
                    pX = ps_p.tile([W65, SEG], F32, tag="pp")
                    for c in range(NCH):
                        nc.tensor.matmul(
                            pX[0:D, :],
                            wp_sb[:, WKW + c * D:WKW + (c + 1) * D],
                            fa_all[:, s * QW + c * SEG:s * QW + (c + 1) * SEG],
                            start=(c == 0),
                            stop=(c == NCH - 1),
                        )
                    nc.vector.tensor_mul(
                        out=QT[:, s * SEG:(s + 1) * SEG],
                        in0=pX[0:D, :],
                        in1=Cb[:],
                    )

                with (
                    tc.tile_pool(name="pse", space="PSUM", bufs=3) as ps_e,
                    tc.tile_pool(name="osb", bufs=6) as opool,
                ):
                    def _main_tile(t, jb):
                        pe_ = ps_e.tile([P, WOUT], F32)
                        for hh in range(2):
                            nc.tensor.matmul(
                                pe_[:, hh * SEG:(hh + 1) * SEG],
                                QT[:, t * P:(t + 1) * P],
                                KpT[:, jb * WOUT + hh * SEG:
                                    jb * WOUT + (hh + 1) * SEG],
                                start=True,
                                stop=True,
                            )
                        osb = opool.tile([P, WOUT], F16)
                        if t % 2 == 0:
                            nc.scalar.activation(osb[:], pe_[:], COPY)
                        else:
                            nc.vector.tensor_copy(out=osb[:], in_=pe_[:])
                        nc.sync.dma_start(
                            out=G[t * P:(t + 1) * P,
                                  jb * WOUT:(jb + 1) * WOUT],
                            in_=osb[:],
                        )

                    # jb=0 with own-half prelims interleaved
                    for s in (0, 1):
                        Cb = _k_group(s)
                        _q_group(s, Cb)
                    for t in range(NIT // 2):
                        _main_tile(t, 0)
                    for s in (2, 3):
                        Cb = _k_group(s)
                        _q_group(s, Cb)
                    for t in range(NIT // 2, NIT):
                        _main_tile(t, 0)
                    # jb=1; prefetch other-half K prelims one phase ahead
                    _k_group(4)
                    _k_group(5)
                    for t in range(NIT):
                        _main_tile(t, 1)
                    # jb=2
                    _k_group(6)
                    _k_group(7)
                    for t in range(NIT):
                        _main_tile(t, 2)
                    # jb=3
                    for t in range(NIT):
                        _main_tile(t, 3)

    nc.compile()
    return nc


def _get_program():
    global _PROGRAM
    if _PROGRAM is None:
        _PROGRAM = _build_program()
    return _PROGRAM


def _make_in_maps(features, Wq, Wk, w_charge, b_charge, loc):
    wk65 = np.concatenate([Wk, w_charge[:, None]], axis=1).astype(np.float16)
    wq64 = (Wq * np.float32(-loc / 8.0)).astype(np.float16)
    wk_r = wk65.reshape(NCH, P, D + 1).transpose(1, 0, 2).reshape(P, WKW)
    wq_r = wq64.reshape(NCH, P, D).transpose(1, 0, 2).reshape(P, WQW)
    bcol = np.full((P, 1), b_charge, dtype=np.float16)
    wpack = np.ascontiguousarray(np.concatenate([wk_r, wq_r, bcol], axis=1))

    halves = []
    for b in range(B):
        fb16 = features[b].astype(np.float16)  # [S, F]
        packs = []
        for h in range(2):
            own = fb16[h * IHALF:(h + 1) * IHALF]          # [2048, 512]
            pk = own.reshape(4, SEG, NCH, P).transpose(3, 0, 2, 1)
            packs.append(np.ascontiguousarray(pk.reshape(P, FPK)))
        halves.append(packs)

    in_maps = []
    for core in range(2 * B):
        b, h = divmod(core, 2)
        in_maps.append({
            "fA": halves[b][h],
            "fB": halves[b][1 - h],
            "wpack": wpack,
        })
    return in_maps


def _host_masks():
    """Toeplitz 1/dist blocks: diagonal [2048,2048] and off-diagonal."""
    idx = np.arange(IHALF, dtype=np.float32)
    md = 1.0 / np.maximum(np.abs(idx[:, None] - idx[None, :]), 1.0)
    mo = 1.0 / (np.float32(IHALF) + idx[None, :] - idx[:, None])
    return md.astype(np.float32), mo.astype(np.float32)


def kernel(features, Wq, Wk, w_charge, b_charge, locality_scale):
    features = np.asarray(features, dtype=np.float32)
    Wq = np.asarray(Wq, dtype=np.float32)
    Wk = np.asarray(Wk, dtype=np.float32)
    w_charge = np.asarray(w_charge, dtype=np.float32)
    b_charge = float(np.asarray(b_charge))
    loc = float(np.asarray(locality_scale))

    nc = _get_program()
    in_maps = _make_in_maps(features, Wq, Wk, w_charge, b_charge, loc)
    res = bass_utils.run_bass_kernel_spmd(nc, in_maps, core_ids=list(range(2 * B)))

    md, mo = _host_masks()
    mot = np.ascontiguousarray(mo.T)
    out = np.empty((B, S, S), dtype=np.float32)
    for core in range(2 * B):
        b, h = divmod(core, 2)
        i0 = h * IHALF
        o0 = (1 - h) * IHALF
        Gc = res.results[core]["G"]
        np.multiply(Gc[:, :IHALF], md, out=out[b, i0:i0 + IHALF, i0:i0 + IHALF])
        np.multiply(Gc[:, IHALF:], mo if h == 0 else mot,
                    out=out[b, i0:i0 + IHALF, o0:o0 + IHALF])
    return out
